# revision 21
# baseline (speedup 1.0000x reference)
"""ConvNeXt block (nn_CNBlock) Trainium2 Bass kernel.

Reference computation (per image, fp32):
  y = depthwise_conv7x7(x, conv_w) + conv_b          # NCHW, pad 3
  y = LayerNorm_channel(y) * ln_g + ln_b             # over C at each pixel
  h = gelu(y^T @ w1 + b1, exact)                     # C -> 4C
  out = h @ w2 + b2                                  # 4C -> C  (NCHW out)

Strategy: data-parallel over batch, 4 images per NeuronCore x 8 cores.
Per core, channels-first fp16 layout [C=2x128 partitions, 3136 compact
pixels].  The 49 conv taps are split per-image between:
  - PE: diagonal-matrix matmuls (fp16, 1 cyc/col) accumulated in PSUM,
    evicted per 448-px chunk by the ACT engine -> ya,
  - DVE: tensor_scalar prescale (fast 1-src mode) + tensor_tensor
    accumulate (2x packed fp16) -> yd; then ya += yd (DVE TT).
LN without centering the activations first:
  sums/sumsq via ones-matmuls (PSUM broadcast); mu, mu^2, E[y^2] evicted
  by ACT as fp16; var = E[y^2]-mu^2 (DVE TT); rsqrt on ACT;
  u = ya-mu (GpSimd TT); yh = u*rsqrt (DVE TT).  MLP: fp16 matmuls on
  PE, exact-erf Gelu + biases on ACT.  fp16 keeps rel err ~1e-3.
"""
import sys

sys.path.insert(0, "/opt/trn_rl_repo")

import numpy as np

# ---------------- problem constants (hardcoded) ----------------
B, DIM, H, W = 32, 256, 56, 56
HID = 4 * DIM
EPS = 1e-6
NCORES = 8
BS = B // NCORES          # images per core
NCT = 2                   # channel tiles of 128
G = 62                    # padded grid width
NPX = H * W               # compact pixels per image 3136
XPL = 3856                # padded input tile length (3 + 62*62 + 9)
NCH = 7                   # pixel chunks
RCH = 8                   # rows per chunk
CW = RCH * W              # chunk width 448

T_PE = [33, 34, 31, 29]   # per-image conv taps on the PE (rest on DVE)
T_MAX = max(T_PE)
T_MIN = min(T_PE)

_CACHE = {}
ALL_TAPS = [(di, dj) for di in range(7) for dj in range(7)]


def _build_program():
    import concourse.bacc as bacc
    import concourse.mybir as mybir
    import concourse.tile as tile

    dt = mybir.dt
    AF = mybir.ActivationFunctionType
    ALU = mybir.AluOpType
    F16 = dt.float16
    F32 = dt.float32

    NTV = 49 - T_MIN      # DVE tap weight columns (union, taps T_MIN..48)

    nc = bacc.Bacc("TRN2", target_bir_lowering=False, debug=False)

    d_xpad = nc.dram_tensor("xpad", [BS, NCT, 128, XPL], F16, kind="ExternalInput")
    d_w1 = nc.dram_tensor("w1sb", [128, 2048], F16, kind="ExternalInput")
    d_w2 = nc.dram_tensor("w2sb", [128, 2048], F16, kind="ExternalInput")
    d_dg = nc.dram_tensor("dgsb", [128, NCT * T_MAX * 128], F16, kind="ExternalInput")
    d_ones = nc.dram_tensor("ones128", [128, 128], F16, kind="ExternalInput")
    # fp32 const columns: 0-1 cb, 2-9 b1eff, 10-11 b2, 12 eps, 13.. DVE tap weights
    NC_CONST = 13 + NCT * NTV
    d_cst = nc.dram_tensor("cstf", [128, NC_CONST], F32, kind="ExternalInput")
    d_out = nc.dram_tensor("yout", [BS, NCT, 128, NPX], F32, kind="ExternalOutput")

    with tile.TileContext(nc) as tc:
        with (
            tc.tile_pool(name="static", bufs=1) as stat,
            tc.tile_pool(name="xp", bufs=4) as p_xp,
            tc.tile_pool(name="yd", bufs=2) as p_yd,
            tc.tile_pool(name="tmp", bufs=1) as p_tmp,
            tc.tile_pool(name="ya", bufs=4) as p_ya,
            tc.tile_pool(name="mu", bufs=1) as p_mu,
            tc.tile_pool(name="msq", bufs=1) as p_msq,
            tc.tile_pool(name="s2", bufs=1) as p_s2,
            tc.tile_pool(name="va", bufs=1) as p_va,
            tc.tile_pool(name="y2", bufs=2) as p_y2,
            tc.tile_pool(name="yh", bufs=2) as p_yh,
            tc.tile_pool(name="hb", bufs=2) as p_h,
            tc.tile_pool(name="outc", bufs=2) as p_out,
            tc.tile_pool(name="pconv", bufs=2, space="PSUM") as ps_conv,
            tc.tile_pool(name="psy", bufs=1, space="PSUM") as ps_sy,
            tc.tile_pool(name="psy2", bufs=1, space="PSUM") as ps_sy2,
            tc.tile_pool(name="ph", bufs=2, space="PSUM") as ps_h,
            tc.tile_pool(name="po", bufs=2, space="PSUM") as ps_o,
        ):
            w1sb = stat.tile([128, 2048], F16, name="w1sb")
            w2sb = stat.tile([128, 2048], F16, name="w2sb")
            dgsb = stat.tile([128, NCT * T_MAX * 128], F16, name="dgsb")
            ones128 = stat.tile([128, 128], F16, name="ones128")
            cst = stat.tile([128, NC_CONST], F32, name="cst")

            xps = {}
            yds = {}
            yas = {}
            y2s = {}
            mus = {}
            msqs = {}
            s2s = {}
            vas = {}
            yhs = {}

            def xview(xp, di, dj, r0, nr):
                # [128, nr, 56] view of the padded grid for a tap at output rows r0..
                off = 3 + di * G + dj + r0 * G
                return xp[:, off: off + (nr - 1) * G + W + 6].rearrange(
                    "p (r g) -> p r g", g=G
                )[:, 0:nr, 0:W]

            def dma_in(b):
                for ct in range(NCT):
                    xp = p_xp.tile([128, XPL], F16, name=f"xp_{b}_{ct}", tag="xp")
                    nc.sync.dma_start(xp[:], d_xpad.ap()[b, ct])
                    xps[(b, ct)] = xp

            def conv_dve(b, ct):
                # DVE taps: tensor_scalar prescale + tensor_tensor accumulate
                xp = xps[(b, ct)]
                yd = p_yd.tile([128, NPX], F16, name=f"yd_{b}_{ct}", tag="yd")
                yds[(b, ct)] = yd
                ydv = yd[:, 0:NPX].rearrange("p (r c) -> p r c", c=W)
                for j in range(T_PE[b], 49):
                    di, dj = ALL_TAPS[j]
                    xv = xview(xp, di, dj, 0, H)
                    wcol = cst[:, 13 + ct * NTV + (j - T_MIN): 14 + ct * NTV + (j - T_MIN)]
                    if j == T_PE[b]:
                        # init with conv bias folded in: yd = x*w + cb
                        nc.vector.tensor_scalar(
                            ydv, xv, wcol, cst[:, ct:ct + 1],
                            op0=ALU.mult, op1=ALU.add,
                        )
                    else:
                        nc.vector.scalar_tensor_tensor(
                            ydv, xv, wcol, ydv, op0=ALU.mult, op1=ALU.add,
                        )

            def conv_pe(b, ct):
                xp = xps[(b, ct)]
                ya = p_ya.tile([128, NPX], F16, name=f"ya_{b}_{ct}", tag="ya")
                yas[(b, ct)] = ya
                t_pe = T_PE[b]
                for ch in range(NCH):
                    pc = ps_conv.tile([128, CW], F32, name=f"pc_{b}_{ct}_{ch}", tag="pc")
                    pcv = pc[:, 0:CW].rearrange("p (r c) -> p r c", c=W)
                    for i in range(t_pe):
                        di, dj = ALL_TAPS[i]
                        nc.tensor.matmul(
                            pcv,
                            dgsb[:, (ct * T_MAX + i) * 128:(ct * T_MAX + i + 1) * 128],
                            xview(xp, di, dj, ch * RCH, RCH),
                            start=(i == 0),
                            stop=(i == t_pe - 1),
                        )
                    nc.scalar.activation(
                        ya[:, ch * CW:(ch + 1) * CW], pc[:], AF.Identity, bias=0.0,
                    )

            def merge_sq(b):
                # ya += yd (DVE); y2 = ya*ya raw (DVE)
                for ct in range(NCT):
                    ya = yas[(b, ct)]
                    nc.vector.tensor_tensor(ya[:], yds[(b, ct)][:], ya[:], op=ALU.add)
                for ct in range(NCT):
                    y2 = p_y2.tile([128, NPX], F16, name=f"y2_{b}_{ct}", tag="y2")
                    y2s[(b, ct)] = y2
                    nc.vector.tensor_tensor(y2[:], yas[(b, ct)][:], yas[(b, ct)][:], op=ALU.mult)

            def alloc_stats(b):
                mus[b] = p_mu.tile([128, NPX], F16, name=f"mu_{b}", tag="mu")
                msqs[b] = p_msq.tile([128, NPX], F16, name=f"msq_{b}", tag="msq")
                s2s[b] = p_s2.tile([128, NPX], F16, name=f"s2_{b}", tag="s2")

            def stats_chunk(b, ch):
                # sums + sumsq matmuls for chunk, evicted to mu/musq/s2 (fp16)
                sl = slice(ch * CW, (ch + 1) * CW)
                psy = ps_sy.tile([128, CW], F32, name=f"psy_{b}_{ch}", tag="psy")
                nc.tensor.matmul(psy[:], ones128[:], yas[(b, 0)][:, sl], start=True, stop=False)
                nc.tensor.matmul(psy[:], ones128[:], yas[(b, 1)][:, sl], start=False, stop=True)
                nc.scalar.activation(mus[b][:, sl], psy[:], AF.Copy, bias=0.0, scale=1.0 / DIM)
                nc.scalar.activation(msqs[b][:, sl], psy[:], AF.Square, bias=0.0, scale=1.0 / DIM)
                psq = ps_sy2.tile([128, CW], F32, name=f"psq_{b}_{ch}", tag="psq")
                nc.tensor.matmul(psq[:], ones128[:], y2s[(b, 0)][:, sl], start=True, stop=False)
                nc.tensor.matmul(psq[:], ones128[:], y2s[(b, 1)][:, sl], start=False, stop=True)
                nc.scalar.activation(s2s[b][:, sl], psq[:], AF.Copy, bias=0.0, scale=1.0 / DIM)

            def ln_tail(b):
                # var = s2 - mu^2 (in place, DVE); va = rsqrt(var+eps) (ACT);
                # u = ya - mu (GpSimd, in place); yh = u * va (DVE)
                s2 = s2s[b]
                nc.vector.tensor_tensor(s2[:], s2[:], msqs[b][:], op=ALU.subtract)
                va = p_va.tile([128, NPX], F16, name=f"va_{b}", tag="va")
                vas[b] = va
                nc.scalar.activation(va[:], s2[:], AF.Abs_reciprocal_sqrt, bias=cst[:, 12:13])
                for ct in range(NCT):
                    ya = yas[(b, ct)]
                    nc.gpsimd.tensor_tensor(ya[:], ya[:], mus[b][:], op=ALU.subtract)
                yh = p_yh.tile([128, NCT * NPX], F16, name=f"yh_{b}", tag="yh")
                yhs[b] = yh
                for ct in range(NCT):
                    nc.vector.tensor_tensor(
                        yh[:, ct * NPX:(ct + 1) * NPX], yas[(b, ct)][:], va[:], op=ALU.mult,
                    )

            def mlp_chunk(b, ch):
                yh = yhs[b]
                sl0 = slice(ch * CW, (ch + 1) * CW)
                sl1 = slice(NPX + ch * CW, NPX + (ch + 1) * CW)
                hb = p_h.tile([128, 8 * CW], F16, name=f"hb_{b}_{ch}", tag="hb")
                for f in range(8):
                    ph = ps_h.tile([128, CW], F32, name=f"ph_{b}_{ch}_{f}", tag="ph")
                    nc.tensor.matmul(
                        ph[:], w1sb[:, f * 128:(f + 1) * 128], yh[:, sl0],
                        start=True, stop=False,
                    )
                    nc.tensor.matmul(
                        ph[:], w1sb[:, 1024 + f * 128:1024 + (f + 1) * 128],
                        yh[:, sl1], start=False, stop=True,
                    )
                    nc.scalar.activation(
                        hb[:, f * CW:(f + 1) * CW], ph[:], AF.Gelu,
                        bias=cst[:, 2 + f:3 + f],
                    )
                oc = p_out.tile([128, 2 * CW], F32, name=f"oc_{b}_{ch}", tag="oc")
                for ct in range(NCT):
                    po = ps_o.tile([128, CW], F32, name=f"po_{b}_{ch}_{ct}", tag="po")
                    for f in range(8):
                        nc.tensor.matmul(
                            po[:], w2sb[:, f * 256 + ct * 128: f * 256 + (ct + 1) * 128],
                            hb[:, f * CW:(f + 1) * CW],
                            start=(f == 0), stop=(f == 7),
                        )
                    nc.scalar.activation(
                        oc[:, ct * CW:(ct + 1) * CW], po[:], AF.Identity,
                        bias=cst[:, 10 + ct:11 + ct],
                    )
                    nc.sync.dma_start(
                        d_out.ap()[b, ct, :, ch * CW:(ch + 1) * CW],
                        oc[:, ct * CW:(ct + 1) * CW],
                    )

            def mlp_stats_interleave(b_mlp, b_stat):
                # PE/Scalar queues alternate mlp chunks (img b_mlp) with
                # stats chunks (img b_stat) to hide PSUM-evict latencies
                if b_stat is not None:
                    alloc_stats(b_stat)
                for ch in range(NCH):
                    if b_mlp is not None:
                        mlp_chunk(b_mlp, ch)
                    if b_stat is not None:
                        stats_chunk(b_stat, ch)

            # ---------------- software pipeline ----------------
            dma_in(0)
            nc.sync.dma_start(cst[:], d_cst.ap())
            nc.sync.dma_start(dgsb[:], d_dg.ap())
            dma_in(1)
            nc.sync.dma_start(w1sb[:], d_w1.ap())
            nc.sync.dma_start(w2sb[:], d_w2.ap())
            nc.sync.dma_start(ones128[:], d_ones.ap())
            for ct in range(NCT):
                conv_dve(0, ct)
            for ct in range(NCT):
                conv_pe(0, ct)
            merge_sq(0)
            conv_dve(1, 0)
            conv_pe(1, 0)
            mlp_stats_interleave(None, 0)
            ln_tail(0)
            conv_dve(1, 1)
            conv_pe(1, 1)
            dma_in(2)

            for b in range(BS):
                if b + 1 < BS:
                    merge_sq(b + 1)
                if b + 2 < BS:
                    conv_dve(b + 2, 0)
                mlp_stats_interleave(b, b + 1 if b + 1 < BS else None)
                if b + 1 < BS:
                    ln_tail(b + 1)
                if b + 2 < BS:
                    conv_dve(b + 2, 1)
                    conv_pe(b + 2, 0)
                    conv_pe(b + 2, 1)
                    if b + 3 < BS:
                        dma_in(b + 3)

    nc.compile()
    return nc


def _host_prep(x, conv_w, conv_b, ln_g, ln_b, w1, b1, w2, b2):
    """Returns (shared static arrays dict, per-core xpad list)."""
    f32 = np.float32
    f16 = np.float16
    x = np.asarray(x, f32)
    conv_w = np.asarray(conv_w, f32)
    conv_b = np.asarray(conv_b, f32)
    ln_g = np.asarray(ln_g, f32)
    ln_b = np.asarray(ln_b, f32)
    w1 = np.asarray(w1, f32)
    b1 = np.asarray(b1, f32)
    w2 = np.asarray(w2, f32)
    b2 = np.asarray(b2, f32)

    NTV = 49 - T_MIN

    # fold LN affine into w1/b1
    w1g = (ln_g[:, None] * w1).astype(f32)                  # [256, 1024]
    b1e = (ln_b @ w1 + b1).astype(f32)                      # [1024]

    # w1sb[c, ct*1024 + f*128 + j] = w1g[ct*128 + c, f*128 + j]
    w1sb = np.ascontiguousarray(
        w1g.reshape(2, 128, 8, 128).transpose(1, 0, 2, 3).reshape(128, 2048)
    ).astype(f16)
    # w2sb[h, f*256 + ct*128 + co] = w2[f*128 + h, ct*128 + co]
    w2sb = np.ascontiguousarray(
        w2.reshape(8, 128, 2, 128).transpose(1, 0, 2, 3).reshape(128, 2048)
    ).astype(f16)
    # diagonal conv matrices for PE taps (union 0..T_MAX-1)
    dgsb = np.zeros((128, NCT * T_MAX * 128), f16)
    idx = np.arange(128)
    for ct in range(NCT):
        for i in range(T_MAX):
            di, dj = ALL_TAPS[i]
            dgsb[idx, (ct * T_MAX + i) * 128 + idx] = conv_w[ct * 128 + idx, 0, di, dj]
    ones128 = np.ones((128, 128), f16)

    NC_CONST = 13 + NCT * NTV
    cst = np.zeros((128, NC_CONST), f32)
    cst[:, 0] = conv_b[:128]
    cst[:, 1] = conv_b[128:]
    cst[:, 2:10] = b1e.reshape(8, 128).T
    cst[:, 10] = b2[:128]
    cst[:, 11] = b2[128:]
    cst[:, 12] = EPS
    for ct in range(NCT):
        for j in range(T_MIN, 49):
            di, dj = ALL_TAPS[j]
            cst[:, 13 + ct * NTV + (j - T_MIN)] = conv_w[ct * 128 + idx, 0, di, dj]

    # padded input grids
    xg = np.zeros((B, DIM, G, G), f16)
    xg[:, :, 3:59, 3:59] = x
    xg = xg.reshape(B, NCT, 128, G * G)
    xpad = np.zeros((B, NCT, 128, XPL), f16)
    xpad[:, :, :, 3:3 + G * G] = xg

    static = dict(w1sb=w1sb, w2sb=w2sb, dgsb=dgsb, ones128=ones128, cstf=cst)
    xpads = [np.ascontiguousarray(xpad[c * BS:(c + 1) * BS]) for c in range(NCORES)]
    return static, xpads


def kernel(**inputs) -> np.ndarray:
    from concourse import bass_utils

    if "nc" not in _CACHE:
        _CACHE["nc"] = _build_program()
    nc = _CACHE["nc"]

    static, xpads = _host_prep(**inputs)
    in_maps = [dict(static, xpad=xpads[c]) for c in range(NCORES)]
    res = bass_utils.run_bass_kernel_spmd(nc, in_maps, core_ids=list(range(NCORES)))

    out = np.empty((B, DIM, H, W), np.float32)
    for c in range(NCORES):
        yo = res.results[c]["yout"].reshape(BS, NCT, 128, H, W)
        for b in range(BS):
            out[c * BS + b, :128] = yo[b, 0]
            out[c * BS + b, 128:] = yo[b, 1]
    return out


# revision 22
# speedup vs baseline: 1.0983x; 1.0983x over previous
"""ConvNeXt block (nn_CNBlock) Trainium2 Bass kernel.

Reference computation (per image, fp32):
  y = depthwise_conv7x7(x, conv_w) + conv_b          # NCHW, pad 3
  y = LayerNorm_channel(y) * ln_g + ln_b             # over C at each pixel
  h = gelu(y^T @ w1 + b1, exact)                     # C -> 4C
  out = h @ w2 + b2                                  # 4C -> C  (NCHW out)

Strategy: data-parallel over batch, 4 images per NeuronCore x 8 cores.
Per core, channels-first fp16 layout [C=2x128 partitions, 3136 compact
pixels].  The 49 conv taps are split per-image between:
  - PE: diagonal-matrix matmuls (fp16, 1 cyc/col) accumulated in PSUM,
    evicted per 448-px chunk by the ACT engine -> ya,
  - DVE: tensor_scalar prescale (fast 1-src mode) + tensor_tensor
    accumulate (2x packed fp16) -> yd; then ya += yd (DVE TT).
LN without centering the activations first:
  sums/sumsq via ones-matmuls (PSUM broadcast); mu, mu^2, E[y^2] evicted
  by ACT as fp16; var = E[y^2]-mu^2 (DVE TT); rsqrt on ACT;
  u = ya-mu (GpSimd TT); yh = u*rsqrt (DVE TT).  MLP: fp16 matmuls on
  PE, exact-erf Gelu + biases on ACT.  fp16 keeps rel err ~1e-3.
"""
import sys

sys.path.insert(0, "/opt/trn_rl_repo")

import numpy as np

# ---------------- problem constants (hardcoded) ----------------
B, DIM, H, W = 32, 256, 56, 56
HID = 4 * DIM
EPS = 1e-6
NCORES = 8
BS = B // NCORES          # images per core
NCT = 2                   # channel tiles of 128
G = 62                    # padded grid width
NPX = H * W               # compact pixels per image 3136
XPL = 3856                # padded input tile length (3 + 62*62 + 9)
NCH = 7                   # pixel chunks
RCH = 8                   # rows per chunk
CW = RCH * W              # chunk width 448

T_PE = [33, 34, 27, 25]   # per-image conv taps on the PE (rest on DVE)
T_MAX = max(T_PE)
T_MIN = min(T_PE)

_CACHE = {}
ALL_TAPS = [(di, dj) for di in range(7) for dj in range(7)]


def _build_program():
    import concourse.bacc as bacc
    import concourse.mybir as mybir
    import concourse.tile as tile

    dt = mybir.dt
    AF = mybir.ActivationFunctionType
    ALU = mybir.AluOpType
    F16 = dt.float16
    F32 = dt.float32

    NTV = 49 - T_MIN      # DVE tap weight columns (union, taps T_MIN..48)

    nc = bacc.Bacc("TRN2", target_bir_lowering=False, debug=False)

    d_xpad = nc.dram_tensor("xpad", [BS, NCT, 128, XPL], F16, kind="ExternalInput")
    d_w1 = nc.dram_tensor("w1sb", [128, 2048], F16, kind="ExternalInput")
    d_w2 = nc.dram_tensor("w2sb", [128, 2048], F16, kind="ExternalInput")
    d_dg = nc.dram_tensor("dgsb", [128, NCT * T_MAX * 128], F16, kind="ExternalInput")
    d_ones = nc.dram_tensor("ones128", [128, 128], F16, kind="ExternalInput")
    # fp32 const columns: 0-1 cb, 2-9 b1eff, 10-11 b2, 12 eps, 13.. DVE tap weights
    NC_CONST = 13 + NCT * NTV
    d_cst = nc.dram_tensor("cstf", [128, NC_CONST], F32, kind="ExternalInput")
    d_out = nc.dram_tensor("yout", [BS, NCT, 128, NPX], F32, kind="ExternalOutput")

    with tile.TileContext(nc) as tc:
        with (
            tc.tile_pool(name="static", bufs=1) as stat,
            tc.tile_pool(name="xp", bufs=4) as p_xp,
            tc.tile_pool(name="yd", bufs=2) as p_yd,
            tc.tile_pool(name="tmp", bufs=1) as p_tmp,
            tc.tile_pool(name="ya", bufs=4) as p_ya,
            tc.tile_pool(name="mu", bufs=1) as p_mu,
            tc.tile_pool(name="msq", bufs=1) as p_msq,
            tc.tile_pool(name="s2", bufs=1) as p_s2,
            tc.tile_pool(name="va", bufs=1) as p_va,
            tc.tile_pool(name="y2", bufs=2) as p_y2,
            tc.tile_pool(name="yh", bufs=2) as p_yh,
            tc.tile_pool(name="hb", bufs=2) as p_h,
            tc.tile_pool(name="outc", bufs=2) as p_out,
            tc.tile_pool(name="pconv", bufs=2, space="PSUM") as ps_conv,
            tc.tile_pool(name="psy", bufs=1, space="PSUM") as ps_sy,
            tc.tile_pool(name="psy2", bufs=1, space="PSUM") as ps_sy2,
            tc.tile_pool(name="ph", bufs=2, space="PSUM") as ps_h,
            tc.tile_pool(name="po", bufs=2, space="PSUM") as ps_o,
        ):
            w1sb = stat.tile([128, 2048], F16, name="w1sb")
            w2sb = stat.tile([128, 2048], F16, name="w2sb")
            dgsb = stat.tile([128, NCT * T_MAX * 128], F16, name="dgsb")
            ones128 = stat.tile([128, 128], F16, name="ones128")
            cst = stat.tile([128, NC_CONST], F32, name="cst")

            xps = {}
            yds = {}
            yas = {}
            y2s = {}
            mus = {}
            msqs = {}
            s2s = {}
            vas = {}
            yhs = {}

            def xview(xp, di, dj, r0, nr):
                # [128, nr, 56] view of the padded grid for a tap at output rows r0..
                off = 3 + di * G + dj + r0 * G
                return xp[:, off: off + (nr - 1) * G + W + 6].rearrange(
                    "p (r g) -> p r g", g=G
                )[:, 0:nr, 0:W]

            def dma_in(b):
                for ct in range(NCT):
                    xp = p_xp.tile([128, XPL], F16, name=f"xp_{b}_{ct}", tag="xp")
                    nc.sync.dma_start(xp[:], d_xpad.ap()[b, ct])
                    xps[(b, ct)] = xp

            def conv_dve(b, ct):
                # DVE taps: tensor_scalar prescale + tensor_tensor accumulate
                xp = xps[(b, ct)]
                yd = p_yd.tile([128, NPX], F16, name=f"yd_{b}_{ct}", tag="yd")
                yds[(b, ct)] = yd
                ydv = yd[:, 0:NPX].rearrange("p (r c) -> p r c", c=W)
                for j in range(T_PE[b], 49):
                    di, dj = ALL_TAPS[j]
                    xv = xview(xp, di, dj, 0, H)
                    wcol = cst[:, 13 + ct * NTV + (j - T_MIN): 14 + ct * NTV + (j - T_MIN)]
                    if j == T_PE[b]:
                        # init with conv bias folded in: yd = x*w + cb
                        nc.vector.tensor_scalar(
                            ydv, xv, wcol, cst[:, ct:ct + 1],
                            op0=ALU.mult, op1=ALU.add,
                        )
                    else:
                        nc.vector.scalar_tensor_tensor(
                            ydv, xv, wcol, ydv, op0=ALU.mult, op1=ALU.add,
                        )

            def conv_pe(b, ct):
                xp = xps[(b, ct)]
                ya = p_ya.tile([128, NPX], F16, name=f"ya_{b}_{ct}", tag="ya")
                yas[(b, ct)] = ya
                t_pe = T_PE[b]
                for ch in range(NCH):
                    pc = ps_conv.tile([128, CW], F32, name=f"pc_{b}_{ct}_{ch}", tag="pc")
                    pcv = pc[:, 0:CW].rearrange("p (r c) -> p r c", c=W)
                    for i in range(t_pe):
                        di, dj = ALL_TAPS[i]
                        nc.tensor.matmul(
                            pcv,
                            dgsb[:, (ct * T_MAX + i) * 128:(ct * T_MAX + i + 1) * 128],
                            xview(xp, di, dj, ch * RCH, RCH),
                            start=(i == 0),
                            stop=(i == t_pe - 1),
                        )
                    nc.scalar.activation(
                        ya[:, ch * CW:(ch + 1) * CW], pc[:], AF.Identity, bias=0.0,
                    )

            def merge_sq(b):
                # ya += yd (DVE); y2 = ya*ya raw (DVE)
                for ct in range(NCT):
                    ya = yas[(b, ct)]
                    nc.vector.tensor_tensor(ya[:], yds[(b, ct)][:], ya[:], op=ALU.add)
                for ct in range(NCT):
                    y2 = p_y2.tile([128, NPX], F16, name=f"y2_{b}_{ct}", tag="y2")
                    y2s[(b, ct)] = y2
                    nc.vector.tensor_tensor(y2[:], yas[(b, ct)][:], yas[(b, ct)][:], op=ALU.mult)

            def alloc_stats(b):
                mus[b] = p_mu.tile([128, NPX], F16, name=f"mu_{b}", tag="mu")
                msqs[b] = p_msq.tile([128, NPX], F16, name=f"msq_{b}", tag="msq")
                s2s[b] = p_s2.tile([128, NPX], F16, name=f"s2_{b}", tag="s2")

            def stats_chunk(b, ch):
                # sums + sumsq matmuls for chunk, evicted to mu/musq/s2 (fp16)
                sl = slice(ch * CW, (ch + 1) * CW)
                psy = ps_sy.tile([128, CW], F32, name=f"psy_{b}_{ch}", tag="psy")
                nc.tensor.matmul(psy[:], ones128[:], yas[(b, 0)][:, sl], start=True, stop=False)
                nc.tensor.matmul(psy[:], ones128[:], yas[(b, 1)][:, sl], start=False, stop=True)
                nc.scalar.activation(mus[b][:, sl], psy[:], AF.Copy, bias=0.0, scale=1.0 / DIM)
                nc.scalar.activation(msqs[b][:, sl], psy[:], AF.Square, bias=0.0, scale=1.0 / DIM)
                psq = ps_sy2.tile([128, CW], F32, name=f"psq_{b}_{ch}", tag="psq")
                nc.tensor.matmul(psq[:], ones128[:], y2s[(b, 0)][:, sl], start=True, stop=False)
                nc.tensor.matmul(psq[:], ones128[:], y2s[(b, 1)][:, sl], start=False, stop=True)
                nc.scalar.activation(s2s[b][:, sl], psq[:], AF.Copy, bias=0.0, scale=1.0 / DIM)

            def ln_tail(b):
                # var = s2 - mu^2 (in place, DVE); va = rsqrt(var+eps) (ACT);
                # u = ya - mu (GpSimd, in place); yh = u * va (DVE)
                s2 = s2s[b]
                nc.vector.tensor_tensor(s2[:], s2[:], msqs[b][:], op=ALU.subtract)
                va = p_va.tile([128, NPX], F16, name=f"va_{b}", tag="va")
                vas[b] = va
                nc.scalar.activation(va[:], s2[:], AF.Abs_reciprocal_sqrt, bias=cst[:, 12:13])
                for ct in range(NCT):
                    ya = yas[(b, ct)]
                    nc.gpsimd.tensor_tensor(ya[:], ya[:], mus[b][:], op=ALU.subtract)
                yh = p_yh.tile([128, NCT * NPX], F16, name=f"yh_{b}", tag="yh")
                yhs[b] = yh
                for ct in range(NCT):
                    nc.vector.tensor_tensor(
                        yh[:, ct * NPX:(ct + 1) * NPX], yas[(b, ct)][:], va[:], op=ALU.mult,
                    )

            def mlp_chunk(b, ch):
                yh = yhs[b]
                sl0 = slice(ch * CW, (ch + 1) * CW)
                sl1 = slice(NPX + ch * CW, NPX + (ch + 1) * CW)
                hb = p_h.tile([128, 8 * CW], F16, name=f"hb_{b}_{ch}", tag="hb")
                for f in range(8):
                    ph = ps_h.tile([128, CW], F32, name=f"ph_{b}_{ch}_{f}", tag="ph")
                    nc.tensor.matmul(
                        ph[:], w1sb[:, f * 128:(f + 1) * 128], yh[:, sl0],
                        start=True, stop=False,
                    )
                    nc.tensor.matmul(
                        ph[:], w1sb[:, 1024 + f * 128:1024 + (f + 1) * 128],
                        yh[:, sl1], start=False, stop=True,
                    )
                    nc.scalar.activation(
                        hb[:, f * CW:(f + 1) * CW], ph[:], AF.Gelu,
                        bias=cst[:, 2 + f:3 + f],
                    )
                oc = p_out.tile([128, 2 * CW], F32, name=f"oc_{b}_{ch}", tag="oc")
                for ct in range(NCT):
                    po = ps_o.tile([128, CW], F32, name=f"po_{b}_{ch}_{ct}", tag="po")
                    for f in range(8):
                        nc.tensor.matmul(
                            po[:], w2sb[:, f * 256 + ct * 128: f * 256 + (ct + 1) * 128],
                            hb[:, f * CW:(f + 1) * CW],
                            start=(f == 0), stop=(f == 7),
                        )
                    nc.scalar.activation(
                        oc[:, ct * CW:(ct + 1) * CW], po[:], AF.Identity,
                        bias=cst[:, 10 + ct:11 + ct],
                    )
                    nc.sync.dma_start(
                        d_out.ap()[b, ct, :, ch * CW:(ch + 1) * CW],
                        oc[:, ct * CW:(ct + 1) * CW],
                    )

            def mlp_stats_interleave(b_mlp, b_stat):
                # PE/Scalar queues alternate mlp chunks (img b_mlp) with
                # stats chunks (img b_stat) to hide PSUM-evict latencies
                if b_stat is not None:
                    alloc_stats(b_stat)
                for ch in range(NCH):
                    if b_mlp is not None:
                        mlp_chunk(b_mlp, ch)
                    if b_stat is not None:
                        stats_chunk(b_stat, ch)

            # ---------------- software pipeline ----------------
            dma_in(0)
            nc.sync.dma_start(cst[:], d_cst.ap())
            nc.sync.dma_start(dgsb[:], d_dg.ap())
            dma_in(1)
            nc.sync.dma_start(w1sb[:], d_w1.ap())
            nc.sync.dma_start(w2sb[:], d_w2.ap())
            nc.sync.dma_start(ones128[:], d_ones.ap())
            for ct in range(NCT):
                conv_dve(0, ct)
            for ct in range(NCT):
                conv_pe(0, ct)
            merge_sq(0)
            conv_dve(1, 0)
            conv_pe(1, 0)
            mlp_stats_interleave(None, 0)
            ln_tail(0)
            conv_dve(1, 1)
            conv_pe(1, 1)
            dma_in(2)

            for b in range(BS):
                if b + 1 < BS:
                    merge_sq(b + 1)
                if b + 2 < BS:
                    conv_dve(b + 2, 0)
                mlp_stats_interleave(b, b + 1 if b + 1 < BS else None)
                if b + 1 < BS:
                    ln_tail(b + 1)
                if b + 2 < BS:
                    conv_dve(b + 2, 1)
                    conv_pe(b + 2, 0)
                    conv_pe(b + 2, 1)
                    if b + 3 < BS:
                        dma_in(b + 3)

    nc.compile()
    return nc


def _host_prep(x, conv_w, conv_b, ln_g, ln_b, w1, b1, w2, b2):
    """Returns (shared static arrays dict, per-core xpad list)."""
    f32 = np.float32
    f16 = np.float16
    x = np.asarray(x, f32)
    conv_w = np.asarray(conv_w, f32)
    conv_b = np.asarray(conv_b, f32)
    ln_g = np.asarray(ln_g, f32)
    ln_b = np.asarray(ln_b, f32)
    w1 = np.asarray(w1, f32)
    b1 = np.asarray(b1, f32)
    w2 = np.asarray(w2, f32)
    b2 = np.asarray(b2, f32)

    NTV = 49 - T_MIN

    # fold LN affine into w1/b1
    w1g = (ln_g[:, None] * w1).astype(f32)                  # [256, 1024]
    b1e = (ln_b @ w1 + b1).astype(f32)                      # [1024]

    # w1sb[c, ct*1024 + f*128 + j] = w1g[ct*128 + c, f*128 + j]
    w1sb = np.ascontiguousarray(
        w1g.reshape(2, 128, 8, 128).transpose(1, 0, 2, 3).reshape(128, 2048)
    ).astype(f16)
    # w2sb[h, f*256 + ct*128 + co] = w2[f*128 + h, ct*128 + co]
    w2sb = np.ascontiguousarray(
        w2.reshape(8, 128, 2, 128).transpose(1, 0, 2, 3).reshape(128, 2048)
    ).astype(f16)
    # diagonal conv matrices for PE taps (union 0..T_MAX-1)
    dgsb = np.zeros((128, NCT * T_MAX * 128), f16)
    idx = np.arange(128)
    for ct in range(NCT):
        for i in range(T_MAX):
            di, dj = ALL_TAPS[i]
            dgsb[idx, (ct * T_MAX + i) * 128 + idx] = conv_w[ct * 128 + idx, 0, di, dj]
    ones128 = np.ones((128, 128), f16)

    NC_CONST = 13 + NCT * NTV
    cst = np.zeros((128, NC_CONST), f32)
    cst[:, 0] = conv_b[:128]
    cst[:, 1] = conv_b[128:]
    cst[:, 2:10] = b1e.reshape(8, 128).T
    cst[:, 10] = b2[:128]
    cst[:, 11] = b2[128:]
    cst[:, 12] = EPS
    for ct in range(NCT):
        for j in range(T_MIN, 49):
            di, dj = ALL_TAPS[j]
            cst[:, 13 + ct * NTV + (j - T_MIN)] = conv_w[ct * 128 + idx, 0, di, dj]

    # padded input grids
    xg = np.zeros((B, DIM, G, G), f16)
    xg[:, :, 3:59, 3:59] = x
    xg = xg.reshape(B, NCT, 128, G * G)
    xpad = np.zeros((B, NCT, 128, XPL), f16)
    xpad[:, :, :, 3:3 + G * G] = xg

    static = dict(w1sb=w1sb, w2sb=w2sb, dgsb=dgsb, ones128=ones128, cstf=cst)
    xpads = [np.ascontiguousarray(xpad[c * BS:(c + 1) * BS]) for c in range(NCORES)]
    return static, xpads


def kernel(**inputs) -> np.ndarray:
    from concourse import bass_utils

    if "nc" not in _CACHE:
        _CACHE["nc"] = _build_program()
    nc = _CACHE["nc"]

    static, xpads = _host_prep(**inputs)
    in_maps = [dict(static, xpad=xpads[c]) for c in range(NCORES)]
    res = bass_utils.run_bass_kernel_spmd(nc, in_maps, core_ids=list(range(NCORES)))

    out = np.empty((B, DIM, H, W), np.float32)
    for c in range(NCORES):
        yo = res.results[c]["yout"].reshape(BS, NCT, 128, H, W)
        for b in range(BS):
            out[c * BS + b, :128] = yo[b, 0]
            out[c * BS + b, 128:] = yo[b, 1]
    return out
